# revision 9
# baseline (speedup 1.0000x reference)
"""Trainium2 Bass kernel for nn_CausalSelfAttention_8237747274097.

Reference math (single-head attention over full n_embd=1024, scale 1/8):
    qkv = x @ W_attn + b_attn ; q,k,v = split(qkv)
    att = softmax(causal(q @ k.T / 8)) ; y = att @ v ; out = y @ W_proj + b_proj

Sharding (8 cores): core c = (batch b = c//2, parity p = c%2). Each core owns 8
of the 16 query row-tiles (128 rows each) of its batch, interleaved/paired so
causal work is balanced, and computes full K/V for the batch. Outputs are
disjoint row slices -> host gather is a pure scatter + bias add.

Math simplifications (all exact):
  - k bias drops out of softmax (constant along the softmax axis after the
    q.bias cross term is absorbed).
  - v bias folds into the output bias: b_eff = b_proj + b_v @ W_proj.
  - 1/8 scale folded into W_q/b_q on the host (exact power of two).
Softmax is computed without max-subtraction (scores are O(3); exp is safe) so
the denominator comes free from a ones-row matmul.

Precision: f16 inputs (x and weights rounded on host); matmuls f16 with fp32
PSUM accumulation. P=exp(S) is f16; num/den share the same rounded P.

Perf structure (CoreSim cost model: 244us vs 282us for the fp32r baseline;
PE busy ~234us at 96% occupancy):
  - all inputs f16, host pre-tiled into layouts giving fully contiguous DMAs
    (halves HBM traffic and descriptor counts; enables FWL weight loads)
  - DMA issue order matches consumption order: first matmul starts after
    ~0.75MB instead of 8MB+ (startup stall 29us -> ~5us)
  - A1 (K^T) runs jc-major to match DMA arrival order
  - softmax denominator off the PE: per-tile partial sums accumulate on DVE,
    cross-partition reduce on (otherwise idle) GpSimd; frees a PSUM bank so
    the projection PSUM (pp) is double-buffered
  - projection matmuls of pair P interleave into pair P+1's S loop to hide
    PSUM-evacuation waits
Rejected routes (measured/modeled): K/V dedup via 2-rank AllGather (collective
costs ~120us modeled, ~170ms in the axon harness; PE saving only 55us), fp8
DoubleRow PV (stationary changes per matmul -> LDWEIGHTS-bound, no net gain),
PSUM region packing (sim accumulation model rejects start=False-first).
"""

import numpy as np
import ml_dtypes

import concourse.bass as bass
import concourse.tile as tile
import concourse.mybir as mybir
from concourse import bacc
from concourse.bass import ts, ds
from concourse import bass_isa
from concourse.bass_utils import run_bass_kernel_spmd

F32 = mybir.dt.float32
F16 = mybir.dt.float16

T, D = 2048, 1024
NT = T // 128          # 16 query/key tiles
DC = D // 128          # 8 contraction chunks
# own query tiles per core parity (descending pairing balances causal work)
OWN = [[15, 12, 11, 8, 7, 4, 3, 0],
       [14, 13, 10, 9, 6, 5, 2, 1]]
CP = [16, 12, 8, 4]    # j-blocks computed per slot-pair (uniform across cores)

_NC_CACHE = {}


def _build(repeat=1, phases=3):
    key = (repeat, phases)
    if key in _NC_CACHE:
        return _NC_CACHE[key]
    nc = bacc.Bacc("TRN2", target_bir_lowering=False, debug=False,
                   enable_asserts=False, num_devices=8)
    # x^T tiled jc-major: col ((jc*8+d)*512 + t') = x[512jc+t', 128d+p]
    xt = nc.dram_tensor("xt", [128, 4 * DC * 512], F16, kind="ExternalInput").ap()
    # own-query x^T tiled ic-major: col ((ic*8+d)*512 + q') -> own q col 512ic+q'
    xq = nc.dram_tensor("xq", [128, 2 * DC * 512], F16, kind="ExternalInput").ap()
    # weights tiled: wk/wq col ((m*8+d)*128 + f) = w[128d+p, 128m+f]
    wk = nc.dram_tensor("wk", [128, DC * DC * 128], F16, kind="ExternalInput").ap()
    wq = nc.dram_tensor("wq", [128, DC * DC * 128], F16, kind="ExternalInput").ap()
    # wv/wp col (d*1024 + f) = w[128d+p, f]
    wv = nc.dram_tensor("wv", [128, DC * 1024], F16, kind="ExternalInput").ap()
    wp = nc.dram_tensor("wp", [128, DC * 1024], F16, kind="ExternalInput").ap()
    bq = nc.dram_tensor("bq", [128, 8], F32, kind="ExternalInput").ap()
    masks = nc.dram_tensor("masks", [16, 128, 256], F16, kind="ExternalInput").ap()
    out = nc.dram_tensor("out", [1024, D], F32, kind="ExternalOutput").ap()
    den_dram = nc.dram_tensor("den_scratch", [1024], F32).ap()

    with tile.TileContext(nc, pool_alloc_mode="queue") as tc:
        def body(_i=None):
            _emit(nc, tc, xt, xq, wk, wq, wv, wp, bq, masks, out, den_dram, phases)
        if repeat == 1:
            body()
        else:
            with tc.For_i(0, repeat, 1):
                body()
    nc.compile()
    _NC_CACHE[key] = nc
    return nc


def _emit(nc, tc, xt_d, xq_d, wk_d, wq_d, wv_d, wp_d, bq_d, masks_d, out, den_dram,
          phases=3):
    with tc.tile_pool(name="pk", bufs=1) as pk_pool, \
         tc.tile_pool(name="pv", bufs=1) as pv_pool, \
         tc.tile_pool(name="pq", bufs=1) as pq_pool, \
         tc.tile_pool(name="small", bufs=1) as small, \
         tc.tile_pool(name="wpp", bufs=1) as wp_pool:

        kT_sb = [pk_pool.tile([128, T], F16, tag=f"k{m}", name=f"kT_sb{m}")
                 for m in range(DC)]
        v_sb = [pv_pool.tile([128, D], F16, tag=f"v{t}", name=f"v_sb{t}")
                for t in range(NT)]
        qT_sb = [[pq_pool.tile([128, 256], F16, tag=f"q{m}_{p}", name=f"qT_sb{m}_{p}")
                  for p in range(4)] for m in range(DC)]
        wp_sb = wp_pool.tile([128, 8192], F16, tag="wp", name="wp_sb")
        bq_sb = small.tile([128, 8], F32, tag="bq", name="bq_sb")
        mask_sb = [small.tile([128, 256], F16, tag=f"mask{i}", name=f"mask{i}")
                   for i in range(16)]

        with tc.tile_pool(name="xw", bufs=1) as xw:
            # -------- DMA prefetch, consumption order --------
            wk_sb = xw.tile([128, 8192], F16, tag="wk", name="wk_sb")
            xt = [xw.tile([128, 4096], F16, tag=f"xt{jc}", name=f"xt{jc}")
                  for jc in range(4)]
            wv_sb = xw.tile([128, 8192], F16, tag="wv", name="wv_sb")
            xq = [xw.tile([128, 4096], F16, tag=f"xq{ic}", name=f"xq{ic}")
                  for ic in range(2)]
            wq_sb = xw.tile([128, 8192], F16, tag="wq", name="wq_sb")

            nc.sync.dma_start(wk_sb[:, 0:1024], wk_d[:, 0:1024])          # m=0
            nc.sync.dma_start(xt[0][:, 0:2048], xt_d[:, 0:2048])          # jc=0 d0-3
            nc.sync.dma_start(xt[0][:, 2048:4096], xt_d[:, 2048:4096])    # jc=0 d4-7
            for m in range(1, 5):
                nc.sync.dma_start(wk_sb[:, ts(m, 1024)], wk_d[:, ts(m, 1024)])
            nc.sync.dma_start(xt[1][:], xt_d[:, ts(1, 4096)])
            for m in range(5, 8):
                nc.sync.dma_start(wk_sb[:, ts(m, 1024)], wk_d[:, ts(m, 1024)])
            for jc in range(2, 4):
                nc.sync.dma_start(xt[jc][:], xt_d[:, ts(jc, 4096)])
            nc.sync.dma_start(wv_sb[:], wv_d[:])
            for ic in range(2):
                nc.sync.dma_start(xq[ic][:], xq_d[:, ts(ic, 4096)])
            nc.sync.dma_start(wq_sb[:], wq_d[:])
            nc.sync.dma_start(wp_sb[:], wp_d[:])
            nc.sync.dma_start(bq_sb[:], bq_d[:])
            for i in range(16):
                nc.sync.dma_start(mask_sb[i][:], masks_d[i, :, :])

            # -------- Phase A1: K^T, jc-major --------
            with tc.tile_pool(name="psA1", bufs=2, space="PSUM") as psA1:
                for jc in range(4):
                    for m in range(DC):
                        ps = psA1.tile([128, 512], F32, tag="A1", name="psA1_t")
                        for d in range(DC):
                            nc.tensor.matmul(ps[:],
                                             wk_sb[:, ds((m * 8 + d) * 128, 128)],
                                             xt[jc][:, ts(d, 512)],
                                             start=(d == 0), stop=(d == DC - 1))
                        nc.scalar.copy(kT_sb[m][:, ts(jc, 512)], ps[:])

            # -------- Phase A0: V, tt-major --------
            with tc.tile_pool(name="psA", bufs=4, space="PSUM") as psA:
                for tt in range(NT):
                    for fc in range(2):
                        ps = psA.tile([128, 512], F32, tag="A", name="psA_t")
                        for d in range(DC):
                            nc.tensor.matmul(
                                ps[:],
                                xt[tt // 4][:, ds(d * 512 + (tt % 4) * 128, 128)],
                                wv_sb[:, ds(d * 1024 + fc * 512, 512)],
                                start=(d == 0), stop=(d == DC - 1))
                        nc.vector.tensor_copy(v_sb[tt][:, ts(fc, 512)], ps[:])

            # -------- Phase A2: Q^T (own rows) --------
            with tc.tile_pool(name="psA2", bufs=2, space="PSUM") as psA2:
                for m in range(DC):
                    ps = psA2.tile([128, 1024], F32, tag="A2", name="psA2_t")
                    for ic in range(2):
                        for d in range(DC):
                            nc.tensor.matmul(ps[:, ts(ic, 512)],
                                             wq_sb[:, ds((m * 8 + d) * 128, 128)],
                                             xq[ic][:, ts(d, 512)],
                                             start=(d == 0), stop=(d == DC - 1))
                    for p in range(4):
                        nc.scalar.activation(qT_sb[m][p][:], ps[:, ts(p, 256)],
                                             mybir.ActivationFunctionType.Identity,
                                             bias=bq_sb[:, m:m + 1])

        if phases <= 1:
            with tc.tile_pool(name="dump", bufs=1) as dump:
                tk = dump.tile([128, 512], F32, tag="tk", name="tk")
                nc.vector.tensor_copy(tk[:], kT_sb[0][:, 0:512])
                nc.sync.dma_start(out[0:128, 0:512], tk[:])
                tq = dump.tile([128, 512], F32, tag="tq", name="tq")
                nc.vector.tensor_copy(tq[:, 0:256], qT_sb[0][0][:])
                nc.sync.dma_start(out[0:128, 512:1024], tq[:])
                tv = dump.tile([128, 512], F32, tag="tv", name="tv")
                nc.vector.tensor_copy(tv[:], v_sb[0][:, 0:512])
                nc.sync.dma_start(out[128:256, 0:512], tv[:])
            return

        # ---------------- Phase B: attention + projection ----------------
        with tc.tile_pool(name="transB", bufs=3) as trans, \
             tc.tile_pool(name="po", bufs=1, space="PSUM") as po_pool, \
             tc.tile_pool(name="psS", bufs=2, space="PSUM") as psS_pool, \
             tc.tile_pool(name="pproj", bufs=2, space="PSUM") as pp_pool:

            # deferred projection-chunk emitters from the previous pair; one
            # chunk is slotted in after each of the first S tiles of this pair
            pending_proj = []

            for P in range(4):
                cp = CP[P]
                dacc = trans.tile([128, 256], F32, tag="dacc", name="dacc_t", bufs=2)
                pts = []
                for tj in range(cp):
                    # S matmuls + exp (+mask) for j-tile tj
                    psS = psS_pool.tile([128, 256], F32, tag="s", name="psS_t")
                    for d in range(DC):
                        nc.tensor.matmul(psS[:],
                                         kT_sb[d][:, ts(tj, 128)],
                                         qT_sb[d][P][:],
                                         start=(d == 0), stop=(d == DC - 1))
                    pt = trans.tile([128, 256], F16, tag=f"pt{tj}", name="pt_t",
                                    bufs=2)
                    nc.scalar.activation(pt[:], psS[:],
                                         mybir.ActivationFunctionType.Exp)
                    mi = tj - (cp - 4)
                    if mi >= 0:
                        nc.vector.tensor_mul(pt[:], pt[:], mask_sb[4 * P + mi][:])
                    pts.append(pt)
                    # denominator partial sum on DVE (PE stays on matmuls)
                    if tj == 0:
                        nc.vector.tensor_copy(dacc[:], pt[:])
                    else:
                        nc.vector.tensor_add(dacc[:], dacc[:], pt[:])
                    # previous pair's projection work fills exp-wait bubbles
                    if pending_proj:
                        pending_proj.pop(0)()
                # cross-partition sum on (idle) GpSimd
                dred = trans.tile([128, 256], F32, tag="dred", name="dred_t", bufs=2)
                nc.gpsimd.partition_all_reduce(dred[:], dacc[:], channels=128,
                                               reduce_op=bass_isa.ReduceOp.add)

                oT = trans.tile([128, 2048], F16, tag="oT", name="oT_t", bufs=2)
                # PV in two d-halves: each accumulation region owns a full PSUM
                # bank (start=True clears has_written bank-wide).
                for half in range(2):
                    po = po_pool.tile([128, 2048], F32, tag="o", name="po_t")
                    for tj in range(cp):
                        for dtl in range(4):
                            dt = 4 * half + dtl
                            nc.tensor.matmul(po[:, ds(512 * dtl, 256)],
                                             v_sb[tj][:, ts(dt, 128)],
                                             pts[tj][:],
                                             start=(tj == 0), stop=(tj == cp - 1))
                    nc.vector.tensor_copy(
                        oT[:, ds(1024 * half, 1024)].rearrange("p (l x) -> p l x", x=256),
                        po[:].rearrange("p (l x) -> p l x", x=512)[:, :, 0:256])

                # denominator -> per-partition reciprocal columns
                nc.sync.dma_start(den_dram[ds(256 * P, 256)], dred[0:1, :])
                den_col = trans.tile([128, 2], F32, tag="dencol", name="den_col")
                nc.sync.dma_start(den_col[:],
                                  den_dram[ds(256 * P, 256)].rearrange("(t p) -> p t", p=128))
                recip = trans.tile([128, 2], F32, tag="recip", name="recip")
                nc.vector.reciprocal(recip[:], den_col[:])

                def make_proj(P, it, fo, oT, recip):
                    def emit():
                        pp = pp_pool.tile([128, 512], F32, tag="pp", name="pp_t")
                        for dt in range(DC):
                            nc.tensor.matmul(pp[:],
                                             oT[:, ds(256 * dt + 128 * it, 128)],
                                             wp_sb[:, ds(dt * 1024 + fo * 512, 512)],
                                             start=(dt == 0), stop=(dt == DC - 1))
                        ob = trans.tile([128, 512], F32, tag="ob", name="ob_t")
                        nc.vector.tensor_scalar_mul(ob[:], pp[:], recip[:, it:it + 1])
                        nc.sync.dma_start(out[ds(128 * (2 * P + it), 128), ts(fo, 512)],
                                          ob[:])
                    return emit

                projs = [make_proj(P, it, fo, oT, recip)
                         for it in range(2) for fo in range(2)]
                if P < 3:
                    # run first chunk now; defer the rest into next pair's S loop
                    projs[0]()
                    pending_proj = projs[1:]
                else:
                    for emit in projs:
                        emit()


def _host_masks(own):
    """(16, 128, 256) f16 multiplicative masks for the last 4 tj of each pair."""
    m = np.zeros((16, 128, 256), np.float32)
    j = np.arange(128)[:, None]
    i = np.arange(128)[None, :]
    for P in range(4):
        cp = CP[P]
        for mi in range(4):
            tj = cp - 4 + mi
            for s in range(2):
                t = own[2 * P + s]
                m[4 * P + mi, :, 128 * s:128 * (s + 1)] = \
                    (128 * tj + j <= 128 * t + i).astype(np.float32)
    return m.astype(np.float16)


def _tile_proj_w(w):
    """[1024,1024] -> [128, 8192] f16, col ((m*8+d)*128+f) = w[128d+p, 128m+f]."""
    return np.ascontiguousarray(
        w.reshape(8, 128, 8, 128).transpose(1, 2, 0, 3).reshape(128, 8192)
    ).astype(np.float16)


def _tile_wide_w(w):
    """[1024,1024] -> [128, 8192] f16, col (d*1024+f) = w[128d+p, f]."""
    return np.ascontiguousarray(
        w.reshape(8, 128, 1024).transpose(1, 0, 2).reshape(128, 8192)
    ).astype(np.float16)


def _tile_x(xT, nchunk):
    """[1024, 512*nchunk] -> [128, 4096*nchunk] f16,
    col ((c*8+d)*512+t') = xT[128d+p, 512c+t']."""
    return np.ascontiguousarray(
        xT.reshape(8, 128, nchunk, 512).transpose(1, 2, 0, 3).reshape(128, -1)
    ).astype(np.float16)


def kernel(x, W_attn, b_attn, W_proj, b_proj, _repeat=1, _results_only=False, _phases=3):
    x = np.asarray(x, np.float32)
    W_attn = np.asarray(W_attn, np.float32)
    b_attn = np.asarray(b_attn, np.float32)
    W_proj = np.asarray(W_proj, np.float32)
    b_proj = np.asarray(b_proj, np.float32)
    B = x.shape[0]

    nc = _build(_repeat, _phases)

    b_eff = (b_proj.astype(np.float64)
             + b_attn[2 * D:].astype(np.float64) @ W_proj.astype(np.float64)
             ).astype(np.float32)
    wq_t = _tile_proj_w(np.ascontiguousarray(W_attn[:, :D]) * np.float32(0.125))
    wk_t = _tile_proj_w(np.ascontiguousarray(W_attn[:, D:2 * D]))
    wv_t = _tile_wide_w(np.ascontiguousarray(W_attn[:, 2 * D:]))
    wp_t = _tile_wide_w(W_proj)
    bqv = (b_attn[:D] * np.float32(0.125)).reshape(8, 128).T
    bqv = np.ascontiguousarray(bqv).astype(np.float32)
    masks_by_par = [_host_masks(OWN[0]), _host_masks(OWN[1])]

    in_maps = []
    for c in range(8):
        b, par = c // 2, c % 2
        own = OWN[par]
        xTb = np.ascontiguousarray(x[b].T)
        cols = np.concatenate([np.arange(128 * t, 128 * (t + 1)) for t in own])
        xqT = np.ascontiguousarray(xTb[:, cols])
        in_maps.append({"xt": _tile_x(xTb, 4), "xq": _tile_x(xqT, 2),
                        "wq": wq_t, "wk": wk_t, "wv": wv_t, "wp": wp_t,
                        "bq": bqv, "masks": masks_by_par[par]})

    res = run_bass_kernel_spmd(nc, in_maps, core_ids=list(range(8)))
    if _results_only:
        return res

    out = np.empty((B, T, D), np.float32)
    for c in range(8):
        b, par = c // 2, c % 2
        part = res.results[c]["out"]
        for s, t in enumerate(OWN[par]):
            out[b, 128 * t:128 * (t + 1), :] = part[128 * s:128 * (s + 1), :] + b_eff
    return out


# revision 10
# speedup vs baseline: 3.3858x; 3.3858x over previous
"""Trainium2 Bass kernel for nn_CausalSelfAttention_8237747274097.

Reference math (single-head attention over full n_embd=1024, scale 1/8):
    qkv = x @ W_attn + b_attn ; q,k,v = split(qkv)
    att = softmax(causal(q @ k.T / 8)) ; y = att @ v ; out = y @ W_proj + b_proj

Sharding (8 cores): core c = (batch b = c//2, parity p = c%2). Each core owns 8
of the 16 query row-tiles (128 rows each) of its batch, interleaved/paired so
causal work is balanced, and computes full K/V for the batch. Outputs are
disjoint row slices -> host gather is a pure scatter + bias add.

Math simplifications (all exact):
  - k bias drops out of softmax (constant along the softmax axis after the
    q.bias cross term is absorbed).
  - v bias folds into the output bias: b_eff = b_proj + b_v @ W_proj.
  - 1/8 scale folded into W_q/b_q on the host (exact power of two).
Softmax is computed without max-subtraction (scores are O(3); exp is safe) so
the denominator comes free from a ones-row matmul.

Precision: f16 inputs (x and weights rounded on host); matmuls f16 with fp32
PSUM accumulation. P=exp(S) is f16; num/den share the same rounded P.

Perf structure (CoreSim cost model: 244us vs 282us for the fp32r baseline;
PE busy ~234us at 96% occupancy):
  - all inputs f16, host pre-tiled into layouts giving fully contiguous DMAs
    (halves HBM traffic and descriptor counts; enables FWL weight loads)
  - DMA issue order matches consumption order: first matmul starts after
    ~0.75MB instead of 8MB+ (startup stall 29us -> ~5us)
  - A1 (K^T) runs jc-major to match DMA arrival order
  - softmax denominator off the PE: per-tile partial sums accumulate on DVE,
    cross-partition reduce on (otherwise idle) GpSimd; frees a PSUM bank so
    the projection PSUM (pp) is double-buffered
  - projection matmuls of pair P interleave into pair P+1's S loop to hide
    PSUM-evacuation waits
Rejected routes (measured/modeled): K/V dedup via 2-rank AllGather (collective
costs ~120us modeled, ~170ms in the axon harness; PE saving only 55us), fp8
DoubleRow PV (stationary changes per matmul -> LDWEIGHTS-bound, no net gain),
PSUM region packing (sim accumulation model rejects start=False-first).
"""

import numpy as np
import ml_dtypes

import concourse.bass as bass
import concourse.tile as tile
import concourse.mybir as mybir
from concourse import bacc
from concourse.bass import ts, ds
from concourse import bass_isa
from concourse.bass_utils import run_bass_kernel_spmd

F32 = mybir.dt.float32
F16 = mybir.dt.float16

T, D = 2048, 1024
NT = T // 128          # 16 query/key tiles
DC = D // 128          # 8 contraction chunks
# own query tiles per core parity (descending pairing balances causal work)
OWN = [[15, 12, 11, 8, 7, 4, 3, 0],
       [14, 13, 10, 9, 6, 5, 2, 1]]
CP = [16, 12, 8, 4]    # j-blocks computed per slot-pair (uniform across cores)

_NC_CACHE = {}


def _build(repeat=1, phases=3):
    key = (repeat, phases)
    if key in _NC_CACHE:
        return _NC_CACHE[key]
    nc = bacc.Bacc("TRN2", target_bir_lowering=False, debug=False,
                   enable_asserts=False, num_devices=8)
    # x^T tiled jc-major: col ((jc*8+d)*512 + t') = x[512jc+t', 128d+p]
    xt = nc.dram_tensor("xt", [128, 4 * DC * 512], F16, kind="ExternalInput").ap()
    # own-query x^T tiled ic-major: col ((ic*8+d)*512 + q') -> own q col 512ic+q'
    xq = nc.dram_tensor("xq", [128, 2 * DC * 512], F16, kind="ExternalInput").ap()
    # weights tiled: wk/wq col ((m*8+d)*128 + f) = w[128d+p, 128m+f]
    wk = nc.dram_tensor("wk", [128, DC * DC * 128], F16, kind="ExternalInput").ap()
    wq = nc.dram_tensor("wq", [128, DC * DC * 128], F16, kind="ExternalInput").ap()
    # wv/wp col (d*1024 + f) = w[128d+p, f]
    wv = nc.dram_tensor("wv", [128, DC * 1024], F16, kind="ExternalInput").ap()
    wp = nc.dram_tensor("wp", [128, DC * 1024], F16, kind="ExternalInput").ap()
    bq = nc.dram_tensor("bq", [128, 8], F32, kind="ExternalInput").ap()
    masks = nc.dram_tensor("masks", [16, 128, 256], F16, kind="ExternalInput").ap()
    out = nc.dram_tensor("out", [1024, D], F32, kind="ExternalOutput").ap()
    den_dram = nc.dram_tensor("den_scratch", [1024], F32).ap()

    with tile.TileContext(nc, pool_alloc_mode="queue") as tc:
        def body(_i=None):
            _emit(nc, tc, xt, xq, wk, wq, wv, wp, bq, masks, out, den_dram, phases)
        if repeat == 1:
            body()
        else:
            with tc.For_i(0, repeat, 1):
                body()
    nc.compile()
    _NC_CACHE[key] = nc
    return nc


def _emit(nc, tc, xt_d, xq_d, wk_d, wq_d, wv_d, wp_d, bq_d, masks_d, out, den_dram,
          phases=3):
    with tc.tile_pool(name="pk", bufs=1) as pk_pool, \
         tc.tile_pool(name="pv", bufs=1) as pv_pool, \
         tc.tile_pool(name="pq", bufs=1) as pq_pool, \
         tc.tile_pool(name="small", bufs=1) as small, \
         tc.tile_pool(name="wpp", bufs=1) as wp_pool:

        kT_sb = [pk_pool.tile([128, T], F16, tag=f"k{m}", name=f"kT_sb{m}")
                 for m in range(DC)]
        v_sb = [pv_pool.tile([128, D], F16, tag=f"v{t}", name=f"v_sb{t}")
                for t in range(NT)]
        qT_sb = [[pq_pool.tile([128, 256], F16, tag=f"q{m}_{p}", name=f"qT_sb{m}_{p}")
                  for p in range(4)] for m in range(DC)]
        wp_sb = wp_pool.tile([128, 8192], F16, tag="wp", name="wp_sb")
        bq_sb = small.tile([128, 8], F32, tag="bq", name="bq_sb")
        mask_sb = [small.tile([128, 256], F16, tag=f"mask{i}", name=f"mask{i}")
                   for i in range(16)]

        with tc.tile_pool(name="xw", bufs=1) as xw:
            # -------- DMA prefetch, consumption order --------
            wk_sb = xw.tile([128, 8192], F16, tag="wk", name="wk_sb")
            xt = [xw.tile([128, 4096], F16, tag=f"xt{jc}", name=f"xt{jc}")
                  for jc in range(4)]
            wv_sb = xw.tile([128, 8192], F16, tag="wv", name="wv_sb")
            xq = [xw.tile([128, 4096], F16, tag=f"xq{ic}", name=f"xq{ic}")
                  for ic in range(2)]
            wq_sb = xw.tile([128, 8192], F16, tag="wq", name="wq_sb")

            nc.sync.dma_start(wk_sb[:, 0:1024], wk_d[:, 0:1024])          # m=0
            nc.sync.dma_start(xt[0][:, 0:2048], xt_d[:, 0:2048])          # jc=0 d0-3
            nc.sync.dma_start(xt[0][:, 2048:4096], xt_d[:, 2048:4096])    # jc=0 d4-7
            for m in range(1, 5):
                nc.sync.dma_start(wk_sb[:, ts(m, 1024)], wk_d[:, ts(m, 1024)])
            nc.sync.dma_start(xt[1][:], xt_d[:, ts(1, 4096)])
            for m in range(5, 8):
                nc.sync.dma_start(wk_sb[:, ts(m, 1024)], wk_d[:, ts(m, 1024)])
            for jc in range(2, 4):
                nc.sync.dma_start(xt[jc][:], xt_d[:, ts(jc, 4096)])
            nc.sync.dma_start(wv_sb[:], wv_d[:])
            for ic in range(2):
                nc.sync.dma_start(xq[ic][:], xq_d[:, ts(ic, 4096)])
            nc.sync.dma_start(wq_sb[:], wq_d[:])
            nc.sync.dma_start(wp_sb[:], wp_d[:])
            nc.sync.dma_start(bq_sb[:], bq_d[:])
            for i in range(16):
                nc.sync.dma_start(mask_sb[i][:], masks_d[i, :, :])

            # -------- Phase A1: K^T, jc-major --------
            with tc.tile_pool(name="psA1", bufs=2, space="PSUM") as psA1:
                for jc in range(4):
                    for m in range(DC):
                        ps = psA1.tile([128, 512], F32, tag="A1", name="psA1_t")
                        for d in range(DC):
                            nc.tensor.matmul(ps[:],
                                             wk_sb[:, ds((m * 8 + d) * 128, 128)],
                                             xt[jc][:, ts(d, 512)],
                                             start=(d == 0), stop=(d == DC - 1))
                        nc.scalar.copy(kT_sb[m][:, ts(jc, 512)], ps[:])

            # -------- Phase A0: V, tt-major --------
            with tc.tile_pool(name="psA", bufs=4, space="PSUM") as psA:
                for tt in range(NT):
                    for fc in range(2):
                        ps = psA.tile([128, 512], F32, tag="A", name="psA_t")
                        for d in range(DC):
                            nc.tensor.matmul(
                                ps[:],
                                xt[tt // 4][:, ds(d * 512 + (tt % 4) * 128, 128)],
                                wv_sb[:, ds(d * 1024 + fc * 512, 512)],
                                start=(d == 0), stop=(d == DC - 1))
                        nc.vector.tensor_copy(v_sb[tt][:, ts(fc, 512)], ps[:])

            # -------- Phase A2: Q^T (own rows) --------
            with tc.tile_pool(name="psA2", bufs=2, space="PSUM") as psA2:
                for m in range(DC):
                    ps = psA2.tile([128, 1024], F32, tag="A2", name="psA2_t")
                    for ic in range(2):
                        for d in range(DC):
                            nc.tensor.matmul(ps[:, ts(ic, 512)],
                                             wq_sb[:, ds((m * 8 + d) * 128, 128)],
                                             xq[ic][:, ts(d, 512)],
                                             start=(d == 0), stop=(d == DC - 1))
                    for p in range(4):
                        nc.scalar.activation(qT_sb[m][p][:], ps[:, ts(p, 256)],
                                             mybir.ActivationFunctionType.Identity,
                                             bias=bq_sb[:, m:m + 1])

        if phases <= 1:
            with tc.tile_pool(name="dump", bufs=1) as dump:
                tk = dump.tile([128, 512], F32, tag="tk", name="tk")
                nc.vector.tensor_copy(tk[:], kT_sb[0][:, 0:512])
                nc.sync.dma_start(out[0:128, 0:512], tk[:])
                tq = dump.tile([128, 512], F32, tag="tq", name="tq")
                nc.vector.tensor_copy(tq[:, 0:256], qT_sb[0][0][:])
                nc.sync.dma_start(out[0:128, 512:1024], tq[:])
                tv = dump.tile([128, 512], F32, tag="tv", name="tv")
                nc.vector.tensor_copy(tv[:], v_sb[0][:, 0:512])
                nc.sync.dma_start(out[128:256, 0:512], tv[:])
            return

        # ---------------- Phase B: attention + projection ----------------
        with tc.tile_pool(name="transB", bufs=3) as trans, \
             tc.tile_pool(name="po", bufs=1, space="PSUM") as po_pool, \
             tc.tile_pool(name="psS", bufs=2, space="PSUM") as psS_pool, \
             tc.tile_pool(name="pproj", bufs=2, space="PSUM") as pp_pool:

            # deferred projection-chunk emitters from the previous pair; one
            # chunk is slotted in after each of the first S tiles of this pair
            pending_proj = []

            for P in range(4):
                cp = CP[P]
                dacc = trans.tile([128, 256], F32, tag="dacc", name="dacc_t", bufs=2)
                pts = []
                for tj in range(cp):
                    # S matmuls + exp (+mask) for j-tile tj
                    psS = psS_pool.tile([128, 256], F32, tag="s", name="psS_t")
                    for d in range(DC):
                        nc.tensor.matmul(psS[:],
                                         kT_sb[d][:, ts(tj, 128)],
                                         qT_sb[d][P][:],
                                         start=(d == 0), stop=(d == DC - 1))
                    pt = trans.tile([128, 256], F16, tag=f"pt{tj}", name="pt_t",
                                    bufs=2)
                    nc.scalar.activation(pt[:], psS[:],
                                         mybir.ActivationFunctionType.Exp)
                    mi = tj - (cp - 4)
                    if mi >= 0:
                        nc.vector.tensor_mul(pt[:], pt[:], mask_sb[4 * P + mi][:])
                    pts.append(pt)
                    # denominator partial sum on DVE (PE stays on matmuls)
                    if tj == 0:
                        nc.vector.tensor_copy(dacc[:], pt[:])
                    else:
                        nc.vector.tensor_add(dacc[:], dacc[:], pt[:])
                    # previous pair's projection work fills exp-wait bubbles
                    if pending_proj:
                        pending_proj.pop(0)()
                # cross-partition sum via XBAR transpose DMA + DVE reduce.
                # f16 for the transpose (2-byte xbar limit); 1/16 scale keeps
                # den (max ~40k) well inside f16 range, folded back below.
                dacc16 = trans.tile([128, 256], F16, tag="dacc16",
                                    name="dacc16_t", bufs=2)
                nc.vector.tensor_scalar_mul(dacc16[:], dacc[:], 0.0625)

                oT = trans.tile([128, 2048], F16, tag="oT", name="oT_t", bufs=2)
                # PV in two d-halves: each accumulation region owns a full PSUM
                # bank (start=True clears has_written bank-wide).
                for half in range(2):
                    po = po_pool.tile([128, 2048], F32, tag="o", name="po_t")
                    for tj in range(cp):
                        for dtl in range(4):
                            dt = 4 * half + dtl
                            nc.tensor.matmul(po[:, ds(512 * dtl, 256)],
                                             v_sb[tj][:, ts(dt, 128)],
                                             pts[tj][:],
                                             start=(tj == 0), stop=(tj == cp - 1))
                    nc.vector.tensor_copy(
                        oT[:, ds(1024 * half, 1024)].rearrange("p (l x) -> p l x", x=256),
                        po[:].rearrange("p (l x) -> p l x", x=512)[:, :, 0:256])

                # denominator -> per-partition reciprocal columns
                den_col = trans.tile([128, 2], F32, tag="dencol", name="den_col")
                for it in range(2):
                    dtr = trans.tile([128, 128], F16, tag="dtr", name="dtr_t",
                                     bufs=2)
                    nc.sync.dma_start_transpose(dtr[:], dacc16[:, ts(it, 128)])
                    nc.vector.reduce_sum(out=den_col[:, it:it + 1], in_=dtr[:],
                                         axis=mybir.AxisListType.X)
                recip = trans.tile([128, 2], F32, tag="recip", name="recip")
                nc.vector.reciprocal(recip[:], den_col[:])
                nc.vector.tensor_scalar_mul(recip[:], recip[:], 0.0625)

                def make_proj(P, it, fo, oT, recip):
                    def emit():
                        pp = pp_pool.tile([128, 512], F32, tag="pp", name="pp_t")
                        for dt in range(DC):
                            nc.tensor.matmul(pp[:],
                                             oT[:, ds(256 * dt + 128 * it, 128)],
                                             wp_sb[:, ds(dt * 1024 + fo * 512, 512)],
                                             start=(dt == 0), stop=(dt == DC - 1))
                        ob = trans.tile([128, 512], F32, tag="ob", name="ob_t")
                        nc.vector.tensor_scalar_mul(ob[:], pp[:], recip[:, it:it + 1])
                        nc.sync.dma_start(out[ds(128 * (2 * P + it), 128), ts(fo, 512)],
                                          ob[:])
                    return emit

                projs = [make_proj(P, it, fo, oT, recip)
                         for it in range(2) for fo in range(2)]
                if P < 3:
                    # run first chunk now; defer the rest into next pair's S loop
                    projs[0]()
                    pending_proj = projs[1:]
                else:
                    for emit in projs:
                        emit()


def _host_masks(own):
    """(16, 128, 256) f16 multiplicative masks for the last 4 tj of each pair."""
    m = np.zeros((16, 128, 256), np.float32)
    j = np.arange(128)[:, None]
    i = np.arange(128)[None, :]
    for P in range(4):
        cp = CP[P]
        for mi in range(4):
            tj = cp - 4 + mi
            for s in range(2):
                t = own[2 * P + s]
                m[4 * P + mi, :, 128 * s:128 * (s + 1)] = \
                    (128 * tj + j <= 128 * t + i).astype(np.float32)
    return m.astype(np.float16)


def _tile_proj_w(w):
    """[1024,1024] -> [128, 8192] f16, col ((m*8+d)*128+f) = w[128d+p, 128m+f]."""
    return np.ascontiguousarray(
        w.reshape(8, 128, 8, 128).transpose(1, 2, 0, 3).reshape(128, 8192)
    ).astype(np.float16)


def _tile_wide_w(w):
    """[1024,1024] -> [128, 8192] f16, col (d*1024+f) = w[128d+p, f]."""
    return np.ascontiguousarray(
        w.reshape(8, 128, 1024).transpose(1, 0, 2).reshape(128, 8192)
    ).astype(np.float16)


def _tile_x(xT, nchunk):
    """[1024, 512*nchunk] -> [128, 4096*nchunk] f16,
    col ((c*8+d)*512+t') = xT[128d+p, 512c+t']."""
    return np.ascontiguousarray(
        xT.reshape(8, 128, nchunk, 512).transpose(1, 2, 0, 3).reshape(128, -1)
    ).astype(np.float16)


def kernel(x, W_attn, b_attn, W_proj, b_proj, _repeat=1, _results_only=False, _phases=3):
    x = np.asarray(x, np.float32)
    W_attn = np.asarray(W_attn, np.float32)
    b_attn = np.asarray(b_attn, np.float32)
    W_proj = np.asarray(W_proj, np.float32)
    b_proj = np.asarray(b_proj, np.float32)
    B = x.shape[0]

    nc = _build(_repeat, _phases)

    b_eff = (b_proj.astype(np.float64)
             + b_attn[2 * D:].astype(np.float64) @ W_proj.astype(np.float64)
             ).astype(np.float32)
    wq_t = _tile_proj_w(np.ascontiguousarray(W_attn[:, :D]) * np.float32(0.125))
    wk_t = _tile_proj_w(np.ascontiguousarray(W_attn[:, D:2 * D]))
    wv_t = _tile_wide_w(np.ascontiguousarray(W_attn[:, 2 * D:]))
    wp_t = _tile_wide_w(W_proj)
    bqv = (b_attn[:D] * np.float32(0.125)).reshape(8, 128).T
    bqv = np.ascontiguousarray(bqv).astype(np.float32)
    masks_by_par = [_host_masks(OWN[0]), _host_masks(OWN[1])]

    in_maps = []
    for c in range(8):
        b, par = c // 2, c % 2
        own = OWN[par]
        xTb = np.ascontiguousarray(x[b].T)
        cols = np.concatenate([np.arange(128 * t, 128 * (t + 1)) for t in own])
        xqT = np.ascontiguousarray(xTb[:, cols])
        in_maps.append({"xt": _tile_x(xTb, 4), "xq": _tile_x(xqT, 2),
                        "wq": wq_t, "wk": wk_t, "wv": wv_t, "wp": wp_t,
                        "bq": bqv, "masks": masks_by_par[par]})

    res = run_bass_kernel_spmd(nc, in_maps, core_ids=list(range(8)))
    if _results_only:
        return res

    out = np.empty((B, T, D), np.float32)
    for c in range(8):
        b, par = c // 2, c % 2
        part = res.results[c]["out"]
        for s, t in enumerate(OWN[par]):
            out[b, 128 * t:128 * (t + 1), :] = part[128 * s:128 * (s + 1), :] + b_eff
    return out


# revision 12
# speedup vs baseline: 3.8187x; 1.1278x over previous
"""Trainium2 Bass kernel for nn_CausalSelfAttention_8237747274097.

Reference math (single-head attention over full n_embd=1024, scale 1/8):
    qkv = x @ W_attn + b_attn ; q,k,v = split(qkv)
    att = softmax(causal(q @ k.T / 8)) ; y = att @ v ; out = y @ W_proj + b_proj

Sharding (8 cores): core c = (batch b = c//2, parity p = c%2). Each core owns 8
of the 16 query row-tiles (128 rows each) of its batch, interleaved/paired so
causal work is balanced, and computes full K/V for the batch. Outputs are
disjoint row slices -> host gather is a pure scatter + bias add.

Math simplifications (all exact):
  - k bias drops out of softmax (constant along the softmax axis after the
    q.bias cross term is absorbed).
  - v bias folds into the output bias: b_eff = b_proj + b_v @ W_proj.
  - 1/8 scale folded into W_q/b_q on the host (exact power of two).
Softmax is computed without max-subtraction (scores are O(3); exp is safe) so
the denominator comes free from a ones-row matmul.

Precision: f16 inputs (x and weights rounded on host); matmuls f16 with fp32
PSUM accumulation. P=exp(S) is f16; num/den share the same rounded P.

Perf structure (CoreSim cost model: 244us vs 282us for the fp32r baseline;
PE busy ~234us at 96% occupancy):
  - all inputs f16, host pre-tiled into layouts giving fully contiguous DMAs
    (halves HBM traffic and descriptor counts; enables FWL weight loads)
  - DMA issue order matches consumption order: first matmul starts after
    ~0.75MB instead of 8MB+ (startup stall 29us -> ~5us)
  - A1 (K^T) runs jc-major to match DMA arrival order
  - softmax denominator off the PE: per-tile partial sums accumulate on DVE,
    cross-partition reduce on (otherwise idle) GpSimd; frees a PSUM bank so
    the projection PSUM (pp) is double-buffered
  - projection matmuls of pair P interleave into pair P+1's S loop to hide
    PSUM-evacuation waits
Rejected routes (measured/modeled): K/V dedup via 2-rank AllGather (collective
costs ~120us modeled, ~170ms in the axon harness; PE saving only 55us), fp8
DoubleRow PV (stationary changes per matmul -> LDWEIGHTS-bound, no net gain),
PSUM region packing (sim accumulation model rejects start=False-first).
"""

import numpy as np
import ml_dtypes

import concourse.bass as bass
import concourse.tile as tile
import concourse.mybir as mybir
from concourse import bacc
from concourse.bass import ts, ds
from concourse import bass_isa
from concourse.bass_utils import run_bass_kernel_spmd

F32 = mybir.dt.float32
F16 = mybir.dt.float16

T, D = 2048, 1024
NT = T // 128          # 16 query/key tiles
DC = D // 128          # 8 contraction chunks
# own query tiles per core parity (descending pairing balances causal work)
OWN = [[15, 12, 11, 8, 7, 4, 3, 0],
       [14, 13, 10, 9, 6, 5, 2, 1]]
CP = [16, 12, 8, 4]    # j-blocks computed per slot-pair (uniform across cores)

_NC_CACHE = {}


def _build(repeat=1, phases=3):
    key = (repeat, phases)
    if key in _NC_CACHE:
        return _NC_CACHE[key]
    nc = bacc.Bacc("TRN2", target_bir_lowering=False, debug=False,
                   enable_asserts=False, num_devices=8)
    # x^T tiled jc-major: col ((jc*8+d)*512 + t') = x[512jc+t', 128d+p]
    xt = nc.dram_tensor("xt", [128, 4 * DC * 512], F16, kind="ExternalInput").ap()
    # own-query x^T tiled d-major: col ((d*2+ic)*512 + q') -> own q col 512ic+q'
    xq = nc.dram_tensor("xq", [128, 2 * DC * 512], F16, kind="ExternalInput").ap()
    # weights tiled: wk/wq col ((m*8+d)*128 + f) = w[128d+p, 128m+f]
    wk = nc.dram_tensor("wk", [128, DC * DC * 128], F16, kind="ExternalInput").ap()
    wq = nc.dram_tensor("wq", [128, DC * DC * 128], F16, kind="ExternalInput").ap()
    # wv/wp col (d*1024 + f) = w[128d+p, f]
    wv = nc.dram_tensor("wv", [128, DC * 1024], F16, kind="ExternalInput").ap()
    wp = nc.dram_tensor("wp", [128, DC * 1024], F16, kind="ExternalInput").ap()
    bq = nc.dram_tensor("bq", [128, 8], F32, kind="ExternalInput").ap()
    masks = nc.dram_tensor("masks", [16, 128, 256], F16, kind="ExternalInput").ap()
    out = nc.dram_tensor("out", [1024, D], F32, kind="ExternalOutput").ap()
    den_dram = nc.dram_tensor("den_scratch", [1024], F32).ap()

    with tile.TileContext(nc, pool_alloc_mode="queue") as tc:
        def body(_i=None):
            _emit(nc, tc, xt, xq, wk, wq, wv, wp, bq, masks, out, den_dram, phases)
        if repeat == 1:
            body()
        else:
            with tc.For_i(0, repeat, 1):
                body()
    nc.compile()
    _NC_CACHE[key] = nc
    return nc


def _emit(nc, tc, xt_d, xq_d, wk_d, wq_d, wv_d, wp_d, bq_d, masks_d, out, den_dram,
          phases=3):
    with tc.tile_pool(name="pk", bufs=1) as pk_pool, \
         tc.tile_pool(name="pv", bufs=1) as pv_pool, \
         tc.tile_pool(name="pq", bufs=1) as pq_pool, \
         tc.tile_pool(name="small", bufs=1) as small, \
         tc.tile_pool(name="wpp", bufs=1) as wp_pool:

        kT_sb = [pk_pool.tile([128, T], F16, tag=f"k{m}", name=f"kT_sb{m}")
                 for m in range(DC)]
        v_sb = [pv_pool.tile([128, D], F16, tag=f"v{t}", name=f"v_sb{t}")
                for t in range(NT)]
        qT_sb = [[pq_pool.tile([128, 256], F16, tag=f"q{m}_{p}", name=f"qT_sb{m}_{p}")
                  for p in range(4)] for m in range(DC)]
        wp_sb = wp_pool.tile([128, 8192], F16, tag="wp", name="wp_sb")
        bq_sb = small.tile([128, 8], F32, tag="bq", name="bq_sb")
        mask_sb = [small.tile([128, 256], F16, tag=f"mask{i}", name=f"mask{i}")
                   for i in range(16)]

        with tc.tile_pool(name="xw", bufs=1) as xw:
            # -------- DMA prefetch, consumption order --------
            wk_sb = xw.tile([128, 8192], F16, tag="wk", name="wk_sb")
            xt_sb = xw.tile([128, 16384], F16, tag="xt", name="xt_sb")
            wv_sb = xw.tile([128, 8192], F16, tag="wv", name="wv_sb")
            xq_sb = xw.tile([128, 8192], F16, tag="xq", name="xq_sb")
            wq_sb = xw.tile([128, 8192], F16, tag="wq", name="wq_sb")

            nc.sync.dma_start(wk_sb[:, 0:1024], wk_d[:, 0:1024])          # m=0
            nc.sync.dma_start(xt_sb[:, 0:2048], xt_d[:, 0:2048])          # jc0 d0-3
            nc.sync.dma_start(xt_sb[:, 2048:4096], xt_d[:, 2048:4096])    # jc0 d4-7
            for m in range(1, 5):
                nc.sync.dma_start(wk_sb[:, ts(m, 1024)], wk_d[:, ts(m, 1024)])
            nc.sync.dma_start(xt_sb[:, ts(1, 4096)], xt_d[:, ts(1, 4096)])  # jc1
            for m in range(5, 8):
                nc.sync.dma_start(wk_sb[:, ts(m, 1024)], wk_d[:, ts(m, 1024)])
            for jc in range(2, 4):
                nc.sync.dma_start(xt_sb[:, ts(jc, 4096)], xt_d[:, ts(jc, 4096)])
            nc.sync.dma_start(wv_sb[:], wv_d[:])
            nc.sync.dma_start(xq_sb[:], xq_d[:])
            nc.sync.dma_start(wq_sb[:], wq_d[:])
            nc.sync.dma_start(wp_sb[:], wp_d[:])
            nc.sync.dma_start(bq_sb[:], bq_d[:])
            for i in range(16):
                nc.sync.dma_start(mask_sb[i][:], masks_d[i, :, :])

            # -------- Phase A1: K^T, jc-major (matches DMA arrival) --------
            with tc.tile_pool(name="psA1", bufs=2, space="PSUM") as psA1:
                for jc in range(4):
                    for m in range(DC):
                        ps = psA1.tile([128, 512], F32, tag="A1", name="psA1_t")
                        for d in range(DC):
                            nc.tensor.matmul(ps[:],
                                             wk_sb[:, ds((m * 8 + d) * 128, 128)],
                                             xt_sb[:, ds((jc * 8 + d) * 512, 512)],
                                             start=(d == 0), stop=(d == DC - 1))
                        nc.scalar.copy(kT_sb[m][:, ts(jc, 512)], ps[:])

            # -------- Phase A0: V; both fc halves share each LDWEIGHTS --------
            with tc.tile_pool(name="psA", bufs=2, space="PSUM") as psA:
                for tt in range(NT):
                    ps2 = [psA.tile([128, 512], F32, tag=f"A_{fc}",
                                    name=f"psA_{fc}") for fc in range(2)]
                    for d in range(DC):
                        for fc in range(2):
                            nc.tensor.matmul(
                                ps2[fc][:],
                                xt_sb[:, ds(((tt // 4) * 8 + d) * 512 + (tt % 4) * 128, 128)],
                                wv_sb[:, ds(d * 1024 + fc * 512, 512)],
                                start=(d == 0), stop=(d == DC - 1))
                    for fc in range(2):
                        nc.vector.tensor_copy(v_sb[tt][:, ts(fc, 512)], ps2[fc][:])

            # -------- Phase A2: Q^T (own rows) --------
            with tc.tile_pool(name="psA2", bufs=2, space="PSUM") as psA2:
                for m in range(DC):
                    ps = psA2.tile([128, 1024], F32, tag="A2", name="psA2_t")
                    for d in range(DC):
                        for ic in range(2):
                            nc.tensor.matmul(ps[:, ts(ic, 512)],
                                             wq_sb[:, ds((m * 8 + d) * 128, 128)],
                                             xq_sb[:, ds((d * 2 + ic) * 512, 512)],
                                             start=(d == 0), stop=(d == DC - 1))
                    for p in range(4):
                        nc.scalar.activation(qT_sb[m][p][:], ps[:, ts(p, 256)],
                                             mybir.ActivationFunctionType.Identity,
                                             bias=bq_sb[:, m:m + 1])

        if phases <= 1:
            with tc.tile_pool(name="dump", bufs=1) as dump:
                tk = dump.tile([128, 512], F32, tag="tk", name="tk")
                nc.vector.tensor_copy(tk[:], kT_sb[0][:, 0:512])
                nc.sync.dma_start(out[0:128, 0:512], tk[:])
                tq = dump.tile([128, 512], F32, tag="tq", name="tq")
                nc.vector.tensor_copy(tq[:, 0:256], qT_sb[0][0][:])
                nc.sync.dma_start(out[0:128, 512:1024], tq[:])
                tv = dump.tile([128, 512], F32, tag="tv", name="tv")
                nc.vector.tensor_copy(tv[:], v_sb[0][:, 0:512])
                nc.sync.dma_start(out[128:256, 0:512], tv[:])
            return

        # ---------------- Phase B: attention + projection ----------------
        with tc.tile_pool(name="transB", bufs=3) as trans, \
             tc.tile_pool(name="po", bufs=1, space="PSUM") as po_pool, \
             tc.tile_pool(name="psS", bufs=2, space="PSUM") as psS_pool, \
             tc.tile_pool(name="pproj", bufs=2, space="PSUM") as pp_pool:

            # deferred projection-chunk emitters from the previous pair; one
            # chunk is slotted in after each of the first S tiles of this pair
            pending_proj = []

            for P in range(4):
                cp = CP[P]
                dacc = trans.tile([128, 256], F32, tag="dacc", name="dacc_t", bufs=2)
                pts = []
                for tj in range(cp):
                    # S matmuls + exp (+mask) for j-tile tj
                    psS = psS_pool.tile([128, 256], F32, tag="s", name="psS_t")
                    for d in range(DC):
                        nc.tensor.matmul(psS[:],
                                         kT_sb[d][:, ts(tj, 128)],
                                         qT_sb[d][P][:],
                                         start=(d == 0), stop=(d == DC - 1))
                    pt = trans.tile([128, 256], F16, tag=f"pt{tj}", name="pt_t",
                                    bufs=2)
                    nc.scalar.activation(pt[:], psS[:],
                                         mybir.ActivationFunctionType.Exp)
                    mi = tj - (cp - 4)
                    if mi >= 0:
                        nc.vector.tensor_mul(pt[:], pt[:], mask_sb[4 * P + mi][:])
                    pts.append(pt)
                    # denominator partial sum on DVE (PE stays on matmuls)
                    if tj == 0:
                        nc.vector.tensor_copy(dacc[:], pt[:])
                    else:
                        nc.vector.tensor_add(dacc[:], dacc[:], pt[:])
                    # previous pair's projection work fills exp-wait bubbles
                    if pending_proj:
                        pending_proj.pop(0)()
                # cross-partition sum via XBAR transpose DMA + DVE reduce.
                # f16 for the transpose (2-byte xbar limit); 1/16 scale keeps
                # den (max ~40k) well inside f16 range, folded back below.
                dacc16 = trans.tile([128, 256], F16, tag="dacc16",
                                    name="dacc16_t", bufs=2)
                nc.vector.tensor_scalar_mul(dacc16[:], dacc[:], 0.0625)

                oT = trans.tile([128, 2048], F16, tag="oT", name="oT_t", bufs=2)
                # PV in two d-halves: each accumulation region owns a full PSUM
                # bank (start=True clears has_written bank-wide).
                for half in range(2):
                    po = po_pool.tile([128, 2048], F32, tag="o", name="po_t")
                    for tj in range(cp):
                        for dtl in range(4):
                            dt = 4 * half + dtl
                            nc.tensor.matmul(po[:, ds(512 * dtl, 256)],
                                             v_sb[tj][:, ts(dt, 128)],
                                             pts[tj][:],
                                             start=(tj == 0), stop=(tj == cp - 1))
                    nc.vector.tensor_copy(
                        oT[:, ds(1024 * half, 1024)].rearrange("p (l x) -> p l x", x=256),
                        po[:].rearrange("p (l x) -> p l x", x=512)[:, :, 0:256])

                # denominator -> per-partition reciprocal columns
                den_col = trans.tile([128, 2], F32, tag="dencol", name="den_col")
                for it in range(2):
                    dtr = trans.tile([128, 128], F16, tag="dtr", name="dtr_t",
                                     bufs=2)
                    nc.sync.dma_start_transpose(dtr[:], dacc16[:, ts(it, 128)])
                    nc.vector.reduce_sum(out=den_col[:, it:it + 1], in_=dtr[:],
                                         axis=mybir.AxisListType.X)
                recip = trans.tile([128, 2], F32, tag="recip", name="recip")
                nc.vector.reciprocal(recip[:], den_col[:])
                nc.vector.tensor_scalar_mul(recip[:], recip[:], 0.0625)

                def make_proj(P, it, fo, oT, recip):
                    def emit():
                        pp = pp_pool.tile([128, 512], F32, tag="pp", name="pp_t")
                        for dt in range(DC):
                            nc.tensor.matmul(pp[:],
                                             oT[:, ds(256 * dt + 128 * it, 128)],
                                             wp_sb[:, ds(dt * 1024 + fo * 512, 512)],
                                             start=(dt == 0), stop=(dt == DC - 1))
                        ob = trans.tile([128, 512], F32, tag="ob", name="ob_t")
                        nc.vector.tensor_scalar_mul(ob[:], pp[:], recip[:, it:it + 1])
                        nc.sync.dma_start(out[ds(128 * (2 * P + it), 128), ts(fo, 512)],
                                          ob[:])
                    return emit

                projs = [make_proj(P, it, fo, oT, recip)
                         for it in range(2) for fo in range(2)]
                if P < 3:
                    # run first chunk now; defer the rest into next pair's S loop
                    projs[0]()
                    pending_proj = projs[1:]
                else:
                    for emit in projs:
                        emit()


def _host_masks(own):
    """(16, 128, 256) f16 multiplicative masks for the last 4 tj of each pair."""
    m = np.zeros((16, 128, 256), np.float32)
    j = np.arange(128)[:, None]
    i = np.arange(128)[None, :]
    for P in range(4):
        cp = CP[P]
        for mi in range(4):
            tj = cp - 4 + mi
            for s in range(2):
                t = own[2 * P + s]
                m[4 * P + mi, :, 128 * s:128 * (s + 1)] = \
                    (128 * tj + j <= 128 * t + i).astype(np.float32)
    return m.astype(np.float16)


def _tile_proj_w(w):
    """[1024,1024] -> [128, 8192] f16, col ((m*8+d)*128+f) = w[128d+p, 128m+f]."""
    return np.ascontiguousarray(
        w.reshape(8, 128, 8, 128).transpose(1, 2, 0, 3).reshape(128, 8192)
    ).astype(np.float16)


def _tile_wide_w(w):
    """[1024,1024] -> [128, 8192] f16, col (d*1024+f) = w[128d+p, f]."""
    return np.ascontiguousarray(
        w.reshape(8, 128, 1024).transpose(1, 0, 2).reshape(128, 8192)
    ).astype(np.float16)


def _tile_x(xT, nchunk, d_major=False):
    """[1024, 512*nchunk] -> [128, 4096*nchunk] f16.
    d_major: col ((d*nchunk+c)*512+t'); else col ((c*8+d)*512+t')."""
    perm = (1, 0, 2, 3) if d_major else (1, 2, 0, 3)
    return np.ascontiguousarray(
        xT.reshape(8, 128, nchunk, 512).transpose(*perm).reshape(128, -1)
    ).astype(np.float16)


def kernel(x, W_attn, b_attn, W_proj, b_proj, _repeat=1, _results_only=False, _phases=3):
    x = np.asarray(x, np.float32)
    W_attn = np.asarray(W_attn, np.float32)
    b_attn = np.asarray(b_attn, np.float32)
    W_proj = np.asarray(W_proj, np.float32)
    b_proj = np.asarray(b_proj, np.float32)
    B = x.shape[0]

    nc = _build(_repeat, _phases)

    b_eff = (b_proj.astype(np.float64)
             + b_attn[2 * D:].astype(np.float64) @ W_proj.astype(np.float64)
             ).astype(np.float32)
    wq_t = _tile_proj_w(np.ascontiguousarray(W_attn[:, :D]) * np.float32(0.125))
    wk_t = _tile_proj_w(np.ascontiguousarray(W_attn[:, D:2 * D]))
    wv_t = _tile_wide_w(np.ascontiguousarray(W_attn[:, 2 * D:]))
    wp_t = _tile_wide_w(W_proj)
    bqv = (b_attn[:D] * np.float32(0.125)).reshape(8, 128).T
    bqv = np.ascontiguousarray(bqv).astype(np.float32)
    masks_by_par = [_host_masks(OWN[0]), _host_masks(OWN[1])]

    in_maps = []
    for c in range(8):
        b, par = c // 2, c % 2
        own = OWN[par]
        xTb = np.ascontiguousarray(x[b].T)
        cols = np.concatenate([np.arange(128 * t, 128 * (t + 1)) for t in own])
        xqT = np.ascontiguousarray(xTb[:, cols])
        in_maps.append({"xt": _tile_x(xTb, 4), "xq": _tile_x(xqT, 2, d_major=True),
                        "wq": wq_t, "wk": wk_t, "wv": wv_t, "wp": wp_t,
                        "bq": bqv, "masks": masks_by_par[par]})

    res = run_bass_kernel_spmd(nc, in_maps, core_ids=list(range(8)))
    if _results_only:
        return res

    out = np.empty((B, T, D), np.float32)
    for c in range(8):
        b, par = c // 2, c % 2
        part = res.results[c]["out"]
        for s, t in enumerate(OWN[par]):
            out[b, 128 * t:128 * (t + 1), :] = part[128 * s:128 * (s + 1), :] + b_eff
    return out


# revision 15
# speedup vs baseline: 3.9623x; 1.0376x over previous
"""Trainium2 Bass kernel for nn_CausalSelfAttention_8237747274097.

Reference math (single-head attention over full n_embd=1024, scale 1/8):
    qkv = x @ W_attn + b_attn ; q,k,v = split(qkv)
    att = softmax(causal(q @ k.T / 8)) ; y = att @ v ; out = y @ W_proj + b_proj

Sharding (8 cores): core c = (batch b = c//2, parity p = c%2). Each core owns 8
of the 16 query row-tiles (128 rows each) of its batch, interleaved/paired so
causal work is balanced, and computes full K/V for the batch. Outputs are
disjoint row slices -> host gather is a pure scatter + bias add.

Math simplifications (all exact):
  - k bias drops out of softmax (constant along the softmax axis after the
    q.bias cross term is absorbed).
  - v bias folds into the output bias: b_eff = b_proj + b_v @ W_proj.
  - 1/8 scale folded into W_q/b_q on the host (exact power of two).
Softmax is computed without max-subtraction (scores are O(3); exp is safe) so
the denominator comes free from a ones-row matmul.

Precision: f16 inputs (x and weights rounded on host); matmuls f16 with fp32
PSUM accumulation. P=exp(S) is f16; num/den share the same rounded P.

Perf structure (CoreSim cost model: 244us vs 282us for the fp32r baseline;
PE busy ~234us at 96% occupancy):
  - all inputs f16, host pre-tiled into layouts giving fully contiguous DMAs
    (halves HBM traffic and descriptor counts; enables FWL weight loads)
  - DMA issue order matches consumption order: first matmul starts after
    ~0.75MB instead of 8MB+ (startup stall 29us -> ~5us)
  - A1 (K^T) runs jc-major to match DMA arrival order
  - softmax denominator off the PE: per-tile partial sums accumulate on DVE,
    cross-partition reduce on (otherwise idle) GpSimd; frees a PSUM bank so
    the projection PSUM (pp) is double-buffered
  - projection matmuls of pair P interleave into pair P+1's S loop to hide
    PSUM-evacuation waits
Rejected routes (measured/modeled): K/V dedup via 2-rank AllGather (collective
costs ~120us modeled, ~170ms in the axon harness; PE saving only 55us), fp8
DoubleRow PV (stationary changes per matmul -> LDWEIGHTS-bound, no net gain),
PSUM region packing (sim accumulation model rejects start=False-first).
"""

import numpy as np
import ml_dtypes

import concourse.bass as bass
import concourse.tile as tile
import concourse.mybir as mybir
from concourse import bacc
from concourse.bass import ts, ds
from concourse.bass_utils import run_bass_kernel_spmd

F32 = mybir.dt.float32
F16 = mybir.dt.float16

T, D = 2048, 1024
NT = T // 128          # 16 query/key tiles
DC = D // 128          # 8 contraction chunks
# own query tiles per core parity (descending pairing balances causal work)
OWN = [[15, 12, 11, 8, 7, 4, 3, 0],
       [14, 13, 10, 9, 6, 5, 2, 1]]
CP = [16, 12, 8, 4]    # j-blocks computed per slot-pair (uniform across cores)

_NC_CACHE = {}


def _build(repeat=1, phases=3):
    key = (repeat, phases)
    if key in _NC_CACHE:
        return _NC_CACHE[key]
    nc = bacc.Bacc("TRN2", target_bir_lowering=False, debug=False,
                   enable_asserts=False, num_devices=8)
    # x^T tiled jc-major: col ((jc*8+d)*512 + t') = x[512jc+t', 128d+p]
    xt = nc.dram_tensor("xt", [128, 4 * DC * 512], F16, kind="ExternalInput").ap()
    # own-query x^T tiled d-major: col ((d*2+ic)*512 + q') -> own q col 512ic+q'
    xq = nc.dram_tensor("xq", [128, 2 * DC * 512], F16, kind="ExternalInput").ap()
    # weights tiled: wk/wq col ((m*8+d)*128 + f) = w[128d+p, 128m+f]
    wk = nc.dram_tensor("wk", [128, DC * DC * 128], F16, kind="ExternalInput").ap()
    wq = nc.dram_tensor("wq", [128, DC * DC * 128], F16, kind="ExternalInput").ap()
    # wv/wp col (d*1024 + f) = w[128d+p, f]
    wv = nc.dram_tensor("wv", [128, DC * 1024], F16, kind="ExternalInput").ap()
    wp = nc.dram_tensor("wp", [128, DC * 1024], F16, kind="ExternalInput").ap()
    bq = nc.dram_tensor("bq", [128, 8], F32, kind="ExternalInput").ap()
    masks = nc.dram_tensor("masks", [16, 128, 256], F16, kind="ExternalInput").ap()
    out = nc.dram_tensor("out", [1024, D], F32, kind="ExternalOutput").ap()

    with tile.TileContext(nc, pool_alloc_mode="queue") as tc:
        def body(_i=None):
            _emit(nc, tc, xt, xq, wk, wq, wv, wp, bq, masks, out, phases)
        if repeat == 1:
            body()
        else:
            with tc.For_i(0, repeat, 1):
                body()
    nc.compile()
    _NC_CACHE[key] = nc
    return nc


def _emit(nc, tc, xt_d, xq_d, wk_d, wq_d, wv_d, wp_d, bq_d, masks_d, out,
          phases=3):
    with tc.tile_pool(name="pk", bufs=1) as pk_pool, \
         tc.tile_pool(name="pv", bufs=1) as pv_pool, \
         tc.tile_pool(name="pq", bufs=1) as pq_pool, \
         tc.tile_pool(name="small", bufs=1) as small, \
         tc.tile_pool(name="wpp", bufs=1) as wp_pool:

        kT_sb = [pk_pool.tile([128, T], F16, tag=f"k{m}", name=f"kT_sb{m}")
                 for m in range(DC)]
        v_sb = [pv_pool.tile([128, D], F16, tag=f"v{t}", name=f"v_sb{t}")
                for t in range(NT)]
        qT_sb = [[pq_pool.tile([128, 256], F16, tag=f"q{m}_{p}", name=f"qT_sb{m}_{p}")
                  for p in range(4)] for m in range(DC)]
        wp_sb = wp_pool.tile([128, 8192], F16, tag="wp", name="wp_sb")
        bq_sb = small.tile([128, 8], F32, tag="bq", name="bq_sb")
        mask_sb = [small.tile([128, 256], F16, tag=f"mask{i}", name=f"mask{i}")
                   for i in range(16)]

        with tc.tile_pool(name="xw", bufs=1) as xw:
            # -------- DMA prefetch, consumption order --------
            wk_sb = xw.tile([128, 8192], F16, tag="wk", name="wk_sb")
            xt_sb = xw.tile([128, 16384], F16, tag="xt", name="xt_sb")
            wv_sb = xw.tile([128, 8192], F16, tag="wv", name="wv_sb")
            xq_sb = xw.tile([128, 8192], F16, tag="xq", name="xq_sb")
            wq_sb = xw.tile([128, 8192], F16, tag="wq", name="wq_sb")

            nc.sync.dma_start(wk_sb[:, 0:1024], wk_d[:, 0:1024])          # m=0
            for q in range(4):                                            # jc0 split
                nc.sync.dma_start(xt_sb[:, ts(q, 1024)], xt_d[:, ts(q, 1024)])
            for m in range(1, 5):
                nc.sync.dma_start(wk_sb[:, ts(m, 1024)], wk_d[:, ts(m, 1024)])
            nc.sync.dma_start(xt_sb[:, ts(1, 4096)], xt_d[:, ts(1, 4096)])  # jc1
            for m in range(5, 8):
                nc.sync.dma_start(wk_sb[:, ts(m, 1024)], wk_d[:, ts(m, 1024)])
            for jc in range(2, 4):
                nc.sync.dma_start(xt_sb[:, ts(jc, 4096)], xt_d[:, ts(jc, 4096)])
            nc.sync.dma_start(wv_sb[:], wv_d[:])
            nc.sync.dma_start(xq_sb[:], xq_d[:])
            nc.sync.dma_start(wq_sb[:], wq_d[:])
            nc.sync.dma_start(wp_sb[:], wp_d[:])
            nc.sync.dma_start(bq_sb[:], bq_d[:])
            for i in range(16):
                nc.sync.dma_start(mask_sb[i][:], masks_d[i, :, :])

            # -------- Phase A1: K^T, jc-major (matches DMA arrival) --------
            with tc.tile_pool(name="psA1", bufs=2, space="PSUM") as psA1:
                for jc in range(4):
                    for m in range(DC):
                        ps = psA1.tile([128, 512], F32, tag="A1", name="psA1_t")
                        for d in range(DC):
                            nc.tensor.matmul(ps[:],
                                             wk_sb[:, ds((m * 8 + d) * 128, 128)],
                                             xt_sb[:, ds((jc * 8 + d) * 512, 512)],
                                             start=(d == 0), stop=(d == DC - 1))
                        nc.scalar.copy(kT_sb[m][:, ts(jc, 512)], ps[:])

            # -------- Phase A0: V; both fc halves share each LDWEIGHTS --------
            with tc.tile_pool(name="psA", bufs=2, space="PSUM") as psA:
                for tt in range(NT):
                    ps2 = [psA.tile([128, 512], F32, tag=f"A_{fc}",
                                    name=f"psA_{fc}") for fc in range(2)]
                    for d in range(DC):
                        for fc in range(2):
                            nc.tensor.matmul(
                                ps2[fc][:],
                                xt_sb[:, ds(((tt // 4) * 8 + d) * 512 + (tt % 4) * 128, 128)],
                                wv_sb[:, ds(d * 1024 + fc * 512, 512)],
                                start=(d == 0), stop=(d == DC - 1))
                    for fc in range(2):
                        nc.vector.tensor_copy(v_sb[tt][:, ts(fc, 512)], ps2[fc][:])

            # -------- Phase A2: Q^T (own rows) --------
            with tc.tile_pool(name="psA2", bufs=2, space="PSUM") as psA2:
                for m in range(DC):
                    ps = psA2.tile([128, 1024], F32, tag="A2", name="psA2_t")
                    for d in range(DC):
                        for ic in range(2):
                            nc.tensor.matmul(ps[:, ts(ic, 512)],
                                             wq_sb[:, ds((m * 8 + d) * 128, 128)],
                                             xq_sb[:, ds((d * 2 + ic) * 512, 512)],
                                             start=(d == 0), stop=(d == DC - 1))
                    for p in range(4):
                        nc.scalar.activation(qT_sb[m][p][:], ps[:, ts(p, 256)],
                                             mybir.ActivationFunctionType.Identity,
                                             bias=bq_sb[:, m:m + 1])

        if phases <= 1:
            with tc.tile_pool(name="dump", bufs=1) as dump:
                tk = dump.tile([128, 512], F32, tag="tk", name="tk")
                nc.vector.tensor_copy(tk[:], kT_sb[0][:, 0:512])
                nc.sync.dma_start(out[0:128, 0:512], tk[:])
                tq = dump.tile([128, 512], F32, tag="tq", name="tq")
                nc.vector.tensor_copy(tq[:, 0:256], qT_sb[0][0][:])
                nc.sync.dma_start(out[0:128, 512:1024], tq[:])
                tv = dump.tile([128, 512], F32, tag="tv", name="tv")
                nc.vector.tensor_copy(tv[:], v_sb[0][:, 0:512])
                nc.sync.dma_start(out[128:256, 0:512], tv[:])
            return

        # ---------------- Phase B: attention + projection ----------------
        with tc.tile_pool(name="transB", bufs=3) as trans, \
             tc.tile_pool(name="po", bufs=1, space="PSUM") as po_pool, \
             tc.tile_pool(name="psS", bufs=2, space="PSUM") as psS_pool, \
             tc.tile_pool(name="pproj", bufs=2, space="PSUM") as pp_pool:

            # deferred projection-chunk emitters from the previous pair; one
            # chunk is slotted in after each of the first S tiles of this pair
            pending_proj = []

            for P in range(4):
                cp = CP[P]
                dacc = trans.tile([128, 256], F32, tag="dacc", name="dacc_t", bufs=2)
                pts = []
                for tj in range(cp):
                    # S matmuls + exp (+mask) for j-tile tj
                    psS = psS_pool.tile([128, 256], F32, tag="s", name="psS_t")
                    for d in range(DC):
                        nc.tensor.matmul(psS[:],
                                         kT_sb[d][:, ts(tj, 128)],
                                         qT_sb[d][P][:],
                                         start=(d == 0), stop=(d == DC - 1))
                    pt = trans.tile([128, 256], F16, tag=f"pt{tj}", name="pt_t",
                                    bufs=2)
                    nc.scalar.activation(pt[:], psS[:],
                                         mybir.ActivationFunctionType.Exp)
                    mi = tj - (cp - 4)
                    if mi >= 0:
                        nc.vector.tensor_mul(pt[:], pt[:], mask_sb[4 * P + mi][:])
                    pts.append(pt)
                    # denominator partial sum on DVE (PE stays on matmuls)
                    if tj == 0:
                        nc.vector.tensor_copy(dacc[:], pt[:])
                    else:
                        nc.vector.tensor_add(dacc[:], dacc[:], pt[:])
                    # previous pair's projection work fills exp-wait bubbles
                    if pending_proj:
                        pending_proj.pop(0)()
                # cross-partition sum via XBAR transpose DMA + DVE reduce.
                # f16 for the transpose (2-byte xbar limit); 1/16 scale keeps
                # den (max ~40k) well inside f16 range, folded back below.
                dacc16 = trans.tile([128, 256], F16, tag="dacc16",
                                    name="dacc16_t", bufs=2)
                nc.vector.tensor_scalar_mul(dacc16[:], dacc[:], 0.0625)

                oT = trans.tile([128, 2048], F16, tag="oT", name="oT_t", bufs=2)
                # PV in two d-halves: each accumulation region owns a full PSUM
                # bank (start=True clears has_written bank-wide).
                for half in range(2):
                    po = po_pool.tile([128, 2048], F32, tag="o", name="po_t")
                    for tj in range(cp):
                        for dtl in range(4):
                            dt = 4 * half + dtl
                            nc.tensor.matmul(po[:, ds(512 * dtl, 256)],
                                             v_sb[tj][:, ts(dt, 128)],
                                             pts[tj][:],
                                             start=(tj == 0), stop=(tj == cp - 1))
                    nc.vector.tensor_copy(
                        oT[:, ds(1024 * half, 1024)].rearrange("p (l x) -> p l x", x=256),
                        po[:].rearrange("p (l x) -> p l x", x=512)[:, :, 0:256])

                # denominator -> per-partition reciprocal columns
                den_col = trans.tile([128, 2], F32, tag="dencol", name="den_col")
                for it in range(2):
                    dtr = trans.tile([128, 128], F16, tag="dtr", name="dtr_t",
                                     bufs=2)
                    nc.sync.dma_start_transpose(dtr[:], dacc16[:, ts(it, 128)])
                    nc.vector.reduce_sum(out=den_col[:, it:it + 1], in_=dtr[:],
                                         axis=mybir.AxisListType.X)
                recip = trans.tile([128, 2], F32, tag="recip", name="recip")
                nc.vector.reciprocal(recip[:], den_col[:])
                nc.vector.tensor_scalar_mul(recip[:], recip[:], 0.0625)

                def make_proj(P, it, fo, oT, recip):
                    def emit():
                        pp = pp_pool.tile([128, 512], F32, tag="pp", name="pp_t")
                        for dt in range(DC):
                            nc.tensor.matmul(pp[:],
                                             oT[:, ds(256 * dt + 128 * it, 128)],
                                             wp_sb[:, ds(dt * 1024 + fo * 512, 512)],
                                             start=(dt == 0), stop=(dt == DC - 1))
                        ob = trans.tile([128, 512], F32, tag="ob", name="ob_t")
                        nc.vector.tensor_scalar_mul(ob[:], pp[:], recip[:, it:it + 1])
                        nc.sync.dma_start(out[ds(128 * (2 * P + it), 128), ts(fo, 512)],
                                          ob[:])
                    return emit

                projs = [make_proj(P, it, fo, oT, recip)
                         for it in range(2) for fo in range(2)]
                if P < 3:
                    # run first chunk now; defer the rest into next pair's S loop
                    projs[0]()
                    pending_proj = projs[1:]
                else:
                    for emit in projs:
                        emit()


def _host_masks(own):
    """(16, 128, 256) f16 multiplicative masks for the last 4 tj of each pair."""
    m = np.zeros((16, 128, 256), np.float32)
    j = np.arange(128)[:, None]
    i = np.arange(128)[None, :]
    for P in range(4):
        cp = CP[P]
        for mi in range(4):
            tj = cp - 4 + mi
            for s in range(2):
                t = own[2 * P + s]
                m[4 * P + mi, :, 128 * s:128 * (s + 1)] = \
                    (128 * tj + j <= 128 * t + i).astype(np.float32)
    return m.astype(np.float16)


def _tile_proj_w(w):
    """[1024,1024] -> [128, 8192] f16, col ((m*8+d)*128+f) = w[128d+p, 128m+f]."""
    return np.ascontiguousarray(
        w.reshape(8, 128, 8, 128).transpose(1, 2, 0, 3).reshape(128, 8192)
    ).astype(np.float16)


def _tile_wide_w(w):
    """[1024,1024] -> [128, 8192] f16, col (d*1024+f) = w[128d+p, f]."""
    return np.ascontiguousarray(
        w.reshape(8, 128, 1024).transpose(1, 0, 2).reshape(128, 8192)
    ).astype(np.float16)


def _tile_x(xT, nchunk, d_major=False):
    """[1024, 512*nchunk] -> [128, 4096*nchunk] f16.
    d_major: col ((d*nchunk+c)*512+t'); else col ((c*8+d)*512+t')."""
    perm = (1, 0, 2, 3) if d_major else (1, 2, 0, 3)
    return np.ascontiguousarray(
        xT.reshape(8, 128, nchunk, 512).transpose(*perm).reshape(128, -1)
    ).astype(np.float16)


def kernel(x, W_attn, b_attn, W_proj, b_proj, _repeat=1, _results_only=False, _phases=3):
    x = np.asarray(x, np.float32)
    W_attn = np.asarray(W_attn, np.float32)
    b_attn = np.asarray(b_attn, np.float32)
    W_proj = np.asarray(W_proj, np.float32)
    b_proj = np.asarray(b_proj, np.float32)
    B = x.shape[0]

    nc = _build(_repeat, _phases)

    b_eff = (b_proj.astype(np.float64)
             + b_attn[2 * D:].astype(np.float64) @ W_proj.astype(np.float64)
             ).astype(np.float32)
    wq_t = _tile_proj_w(np.ascontiguousarray(W_attn[:, :D]) * np.float32(0.125))
    wk_t = _tile_proj_w(np.ascontiguousarray(W_attn[:, D:2 * D]))
    wv_t = _tile_wide_w(np.ascontiguousarray(W_attn[:, 2 * D:]))
    wp_t = _tile_wide_w(W_proj)
    bqv = (b_attn[:D] * np.float32(0.125)).reshape(8, 128).T
    bqv = np.ascontiguousarray(bqv).astype(np.float32)
    masks_by_par = [_host_masks(OWN[0]), _host_masks(OWN[1])]

    in_maps = []
    for c in range(8):
        b, par = c // 2, c % 2
        own = OWN[par]
        xTb = np.ascontiguousarray(x[b].T)
        cols = np.concatenate([np.arange(128 * t, 128 * (t + 1)) for t in own])
        xqT = np.ascontiguousarray(xTb[:, cols])
        in_maps.append({"xt": _tile_x(xTb, 4), "xq": _tile_x(xqT, 2, d_major=True),
                        "wq": wq_t, "wk": wk_t, "wv": wv_t, "wp": wp_t,
                        "bq": bqv, "masks": masks_by_par[par]})

    res = run_bass_kernel_spmd(nc, in_maps, core_ids=list(range(8)))
    if _results_only:
        return res

    out = np.empty((B, T, D), np.float32)
    for c in range(8):
        b, par = c // 2, c % 2
        part = res.results[c]["out"]
        for s, t in enumerate(OWN[par]):
            out[b, 128 * t:128 * (t + 1), :] = part[128 * s:128 * (s + 1), :] + b_eff
    return out
